# revision 26
# baseline (speedup 1.0000x reference)
"""Trainium2 Bass kernel for nn_AttentionTE_15221364097676  (v2).

Reference computation (fp32):
    xn  = LayerNorm(x) * ln_w + ln_b
    qkv = xn @ w_qkv.T -> per-head q,k,v (H=16 heads, C=64), q *= C**-0.5
    a   = softmax(q k^T + bias, masked over keys)
    y   = (a @ v).reshape(B,N,D)
    out = (sigmoid(xn @ w_g.T + b_g) * y) @ w_o.T + b_o

Sharding (8 cores): data-parallel over B (cores 0-3 -> b=0, 4-7 -> b=1),
tensor-parallel over heads (4 heads/core, 2 pairs of 2).  o_proj is
row-parallel; the 4 partial outputs per batch are summed on the host
(+ b_o).

Key ideas vs the v1 baseline (546 us -> target ~2x+):
  * KEY COMPACTION: masked keys contribute exactly 0 to softmax(y), so the
    host reorders keys (a pure gather) putting unmasked keys first; the
    kernel only computes scores/exp/av over ceil(KU/128) key tiles (~half).
    Pad keys use the additive exp bias (-1e30 -> p=0).
  * bf16 datapath: x, weights, q/k, v, p, bias, yg all bf16 (PSUM f32).
  * LN folded into projection copy-out: projections run on RAW x
    (no dependency on LN stats); per-token rstd is applied by DVE at
    PSUM->SBUF copy-out; the mean term is a rank-1 PSUM correction
    (-colsum(W) x mu) like v1.  kv-side stats are computed separately on
    the gathered x_kv.
  * bias injection moved OFF the PE: DVE/GPSIMD copy the bf16 bias tile
    into PSUM (f32) and the score matmuls accumulate on top (start=False).
  * score matmuls for the 2 heads of a pair co-execute on disjoint
    64-partition row groups (A: 0-63, B: 64-127).
  * head B's av output is placed at partitions 63..127 (den@63) so the
    gate multiply and yg write are partition-aligned - no SBUF-SBUF DMAs.
  * epilogue uses DVE divide (g / den) - no ACT Ln/Exp table ping-pong.
  * o_proj is emitted per 512-token chunk right after both pairs'
    epilogues, so it overlaps the next chunk's attention.
"""

import sys

for _p in ("/opt/trn_rl_repo",):
    if _p not in sys.path:
        sys.path.insert(0, _p)

from contextlib import ExitStack

import ml_dtypes
import numpy as np

import concourse.bass as bass
import concourse.tile as tile
from concourse import bacc, mybir
from concourse.bass import ds, ts

F32 = mybir.dt.float32
F32R = mybir.dt.float32r
BF16 = mybir.dt.bfloat16
AF = mybir.ActivationFunctionType
OP = mybir.AluOpType

B, N, D, H, C = 2, 2048, 1024, 16, 64
HPC = 4          # heads per core
NCORES = 8
DT = D // 128    # 8 d-tiles
NT = N // 128    # 16 token tiles
QC = 4           # 512-wide q chunks
EPS = 1e-5
NEG = -1.0e30    # additive key-mask value


def _chunks(total, width=512):
    return [(i * width, min(width, total - i * width))
            for i in range((total + width - 1) // width)]


def _emit(tc, ctx, io, KT, aug):
    nc = tc.nc
    KVN = KT * 128
    kvch = _chunks(KVN)
    KC = len(kvch)

    # ---- long-lived pools ---------------------------------------------------
    const = ctx.enter_context(tc.tile_pool(name="const", bufs=1))
    r_pool = ctx.enter_context(tc.tile_pool(name="rstd", bufs=1))
    qk_pool = ctx.enter_context(tc.tile_pool(name="qkT", bufs=1))
    v_pool = ctx.enter_context(tc.tile_pool(name="v2", bufs=1))
    g_pool = ctx.enter_context(tc.tile_pool(name="gate", bufs=1))
    yg_pool = ctx.enter_context(tc.tile_pool(name="yg", bufs=1))

    # ---- constants ----------------------------------------------------------
    ones_bf = const.tile([128, 128], BF16)
    nc.vector.memset(ones_bf[:], 1.0)
    ones_f = const.tile([128, 128], F32)
    nc.vector.memset(ones_f[:], 1.0)
    ones_fr = const.tile([128, 128], F32R)
    nc.vector.tensor_copy(ones_fr[:], ones_f[:])
    identf = const.tile([128, 128], F32)
    nc.sync.dma_start(identf[:], io["identf"])
    id_bf = const.tile([128, 128], BF16)
    nc.sync.dma_start(id_bf[:], io["identb"])
    wo_sb = const.tile([128, 2, 1024], BF16)
    nc.sync.dma_start(wo_sb[:], io["wo"].rearrange("(t p) e -> p t e", p=128))
    ml_sb = const.tile([128, KT], F32)
    nc.sync.dma_start(ml_sb[:], io["ml"])
    ws_sb = const.tile([128, 1024], F32R)
    nc.sync.dma_start(ws_sb[:], io["ws"])
    bg_sb = const.tile([128, 2], F32)
    nc.sync.dma_start(bg_sb[:], io["bg"])
    eps_sb = const.tile([128, 1], F32)
    nc.vector.memset(eps_sb[:], EPS)
    if aug:
        qb_sb = const.tile([128, 2], F32)
        nc.sync.dma_start(qb_sb[:], io["qb"])
        kb_sb = const.tile([128, 2], F32)
        nc.sync.dma_start(kb_sb[:], io["kb"])
        vbr_sb = const.tile([1, 256], F32R)
        nc.sync.dma_start(vbr_sb[:], io["vb"])

    # per-token rstd (f32), broadcast to 128 partitions / token-major
    rstd_b = r_pool.tile([128, N], F32)
    rstd_kvb = r_pool.tile([128, KVN], F32)
    rstd_tok = r_pool.tile([128, KT], F32)

    qT = qk_pool.tile([128, 2, N], BF16)        # [c-dims(A|B), pair, q-token]
    kT = qk_pool.tile([128, 2, KVN], BF16)      # [c-dims(A|B), pair, k-token]
    v2 = v_pool.tile([128, KT, 2, 130], BF16)   # [k, kt, pair, vA|1|vB|1]
    nc.vector.memset(v2[:], 1.0)
    g_sb = g_pool.tile([128, 2, N], F32)        # [gcol(A|B), pair, token]
    gB_sb = g_pool.tile([128, 2, N], F32)       # head-B gate at partitions 0:64
    yg = yg_pool.tile([128, 2, N], BF16)        # [ydim, pair, token]

    with tc.tile_pool(name="xt", bufs=1) as xpool, \
         tc.tile_pool(name="wts", bufs=1) as wts, \
         tc.tile_pool(name="rows", bufs=1) as rows:

        xt = xpool.tile([128, DT, N], BF16)
        xTr = io["xT"].rearrange("(dt p) n -> p dt n", p=128)
        for dt in range(DT):
            nc.sync.dma_start(xt[:, dt, :], xTr[:, dt, :])
        xkv = xpool.tile([128, DT, KVN], BF16)
        xkvr = io["xkvT"].rearrange("(dt p) n -> p dt n", p=128)
        for dt in range(DT):
            nc.sync.dma_start(xkv[:, dt, :], xkvr[:, dt, :])

        wq_sb = wts.tile([128, DT, 256], BF16)
        nc.sync.dma_start(wq_sb[:], io["wq"].rearrange("(dt p) m -> p dt m", p=128))
        wk_sb = wts.tile([128, DT, 256], BF16)
        nc.sync.dma_start(wk_sb[:], io["wk"].rearrange("(dt p) m -> p dt m", p=128))
        wv_sb = wts.tile([128, DT, 256], BF16)
        nc.sync.dma_start(wv_sb[:], io["wv"].rearrange("(dt p) m -> p dt m", p=128))
        wg_sb = wts.tile([128, DT, 256], BF16)
        nc.sync.dma_start(wg_sb[:], io["wg"].rearrange("(dt p) m -> p dt m", p=128))

        # stat rows: chunk c of 512 tokens lives at partition 64*(c%2),
        # column block c//2 (legal matmul base partitions are {0,32,64});
        # unused partitions are memset so the ln/exp transform stays finite.
        mu_q = rows.tile([128, 2, 512], F32R)
        s2_q = rows.tile([128, 2, 512], F32R)
        mu_kv = rows.tile([128, 2, 512], F32R)
        s2_kv = rows.tile([128, 2, 512], F32R)
        mu2t = rows.tile([128, 2, 512], F32)
        nc.vector.memset(mu2t[:], 1.0)
        nc.vector.tensor_copy(s2_q[:], mu2t[:])
        nc.vector.tensor_copy(s2_kv[:], mu2t[:])
        nc.vector.memset(mu2t[:], 0.0)
        nc.vector.tensor_copy(mu_q[:], mu2t[:])
        nc.vector.tensor_copy(mu_kv[:], mu2t[:])
        rstd_q = s2_q     # transformed in place
        rstd_kv = s2_kv

        # ---- Phase 1: LN stats (sum / sum-sq via ones-matmuls) -------------
        with tc.tile_pool(name="lnps", bufs=2, space="PSUM") as lnps, \
             tc.tile_pool(name="sq", bufs=3) as sqp, \
             tc.tile_pool(name="bps", bufs=2, space="PSUM") as bps_pool:
            for x_t, chl, mu_t, s2_t in ((xt, _chunks(N), mu_q, s2_q),
                                         (xkv, kvch, mu_kv, s2_kv)):
                for c, (off, w) in enumerate(chl):
                    p0, cb = 64 * (c % 2), c // 2
                    sp = lnps.tile([1, 512], F32, tag="sprow")
                    for dt in range(DT):
                        nc.tensor.matmul(sp[:, :w], ones_bf[:, 0:1],
                                         x_t[:, dt, ds(off, w)],
                                         start=(dt == 0), stop=(dt == DT - 1))
                    nc.scalar.activation(mu_t[p0:p0 + 1, cb, :w], sp[:, :w],
                                         AF.Copy, scale=1.0 / D)
                for c, (off, w) in enumerate(chl):
                    p0, cb = 64 * (c % 2), c // 2
                    sp = lnps.tile([1, 512], F32, tag="sprow")
                    for dt in range(DT):
                        sq = sqp.tile([128, 512], BF16, name="sq")
                        nc.vector.tensor_tensor(out=sq[:, :w],
                                                in0=x_t[:, dt, ds(off, w)],
                                                in1=x_t[:, dt, ds(off, w)],
                                                op=OP.mult)
                        nc.tensor.matmul(sp[:, :w], ones_bf[:, 0:1], sq[:, :w],
                                         start=(dt == 0), stop=(dt == DT - 1))
                    nc.scalar.activation(s2_t[p0:p0 + 1, cb, :w], sp[:, :w],
                                         AF.Copy, scale=1.0 / D)
            # var = s2 - mu^2 ; rstd = exp(-0.5*ln(var+eps))    [rows]
            for mu_t, s2_t in ((mu_q, s2_q), (mu_kv, s2_kv)):
                nc.vector.tensor_tensor(out=mu2t[:], in0=mu_t[:], in1=mu_t[:],
                                        op=OP.mult)
                nc.vector.tensor_tensor(out=s2_t[:], in0=s2_t[:], in1=mu2t[:],
                                        op=OP.subtract)
                nc.scalar.activation(s2_t[:], s2_t[:], AF.Ln, bias=eps_sb[:],
                                     scale=1.0)
                nc.scalar.activation(s2_t[:], s2_t[:], AF.Exp, scale=-0.5)

            # broadcast rstd rows across 128 partitions (PE outer products)
            for rs_t, bdst, chl in ((rstd_q, rstd_b, _chunks(N)),
                                    (rstd_kv, rstd_kvb, kvch)):
                for c, (off, w) in enumerate(chl):
                    p0, cb = 64 * (c % 2), c // 2
                    bp = bps_pool.tile([128, 512], F32, tag="bps")
                    nc.tensor.matmul(bp[:, :w], ones_fr[p0:p0 + 1, 0:128],
                                     rs_t[p0:p0 + 1, cb, :w],
                                     start=True, stop=True)
                    nc.vector.tensor_copy(bdst[:, ds(off, w)], bp[:, :w])
            # token-major rstd for the v copy-out (PE transpose of bcast tile)
            for t in range(KT):
                tp = bps_pool.tile([128, 128], F32, tag="tp")
                nc.tensor.transpose(tp[:], rstd_kvb[:, ds(t * 128, 128)],
                                    identf[:])
                nc.vector.tensor_copy(rstd_tok[:, t:t + 1], tp[:, 0:1])
            if aug:
                vb_b = wts.tile([128, 256], F32)
                bp = bps_pool.tile([128, 512], F32, tag="bps")
                nc.tensor.matmul(bp[:, 0:256], ones_fr[0:1, 0:128],
                                 vbr_sb[:], start=True, stop=True)
                nc.vector.tensor_copy(vb_b[:], bp[:, 0:256])

        # ---- Phase 2: projections on RAW x; LN applied at copy-out ----------
        with tc.tile_pool(name="qps", bufs=5, space="PSUM") as qps, \
             tc.tile_pool(name="gtmp", bufs=2) as gtmp:
            # q / k (shared-stationary ordering: dt outer, chunks inner)
            for w_t, x_t, dst, chl, woff, badd in (
                    (wq_sb, xt, qT, _chunks(N), 0, "qb"),
                    (wk_sb, xkv, kT, kvch, 256, "kb")):
                rb_t = rstd_b if dst is qT else rstd_kvb
                mu_t = mu_q if dst is qT else mu_kv
                for mt in range(2):
                    pss = [qps.tile([128, 512], F32, tag="proj", name=f"pp{c}")
                           for c in range(len(chl))]
                    for dt in range(DT):
                        for c, (off, w) in enumerate(chl):
                            nc.tensor.matmul(pss[c][:, :w],
                                             w_t[:, dt, ts(mt, 128)],
                                             x_t[:, dt, ds(off, w)],
                                             start=(dt == 0), stop=False)
                    for c, (off, w) in enumerate(chl):
                        p0, cb = 64 * (c % 2), c // 2
                        nc.tensor.matmul(pss[c][:, :w],
                                         ws_sb[p0:p0 + 1, ds(woff + mt * 128, 128)],
                                         mu_t[p0:p0 + 1, cb, :w],
                                         start=False, stop=True)
                        nc.vector.tensor_tensor(out=dst[:, mt, ds(off, w)],
                                                in0=pss[c][:, :w],
                                                in1=rb_t[:, ds(off, w)],
                                                op=OP.mult)
                        if aug:
                            b_sb = qb_sb if badd == "qb" else kb_sb
                            nc.vector.tensor_scalar(
                                out=dst[:, mt, ds(off, w)],
                                in0=dst[:, mt, ds(off, w)],
                                scalar1=b_sb[:, mt:mt + 1], scalar2=None,
                                op0=OP.add)

            # v: out[token, vcol]; stationary = x tile (changes per mm)
            for t in range(KT):
                ps = qps.tile([128, 256], F32, tag="vproj", bufs=2)
                for dt in range(DT):
                    nc.tensor.matmul(ps[:], xkv[:, dt, ts(t, 128)],
                                     wv_sb[:, dt, :],
                                     start=(dt == 0), stop=False)
                c, off = (t * 128) // 512, (t * 128) % 512
                p0, cb = 64 * (c % 2), c // 2
                nc.tensor.matmul(ps[:], mu_kv[p0:p0 + 1, cb, ds(off, 128)],
                                 ws_sb[p0:p0 + 1, 512:768],
                                 start=False, stop=True)
                for p in range(2):
                    nc.vector.tensor_scalar(
                        out=v2[:, t, p, 0:64], in0=ps[:, ds(p * 128, 64)],
                        scalar1=rstd_tok[:, t:t + 1], scalar2=None, op0=OP.mult)
                    nc.vector.tensor_scalar(
                        out=v2[:, t, p, 65:129], in0=ps[:, ds(p * 128 + 64, 64)],
                        scalar1=rstd_tok[:, t:t + 1], scalar2=None, op0=OP.mult)
                    if aug:
                        nc.vector.tensor_tensor(
                            out=v2[:, t, p, 0:64], in0=v2[:, t, p, 0:64],
                            in1=vb_b[:, ds(p * 128, 64)], op=OP.add)
                        nc.vector.tensor_tensor(
                            out=v2[:, t, p, 65:129], in0=v2[:, t, p, 65:129],
                            in1=vb_b[:, ds(p * 128 + 64, 64)], op=OP.add)

            # gate: sigmoid((Wg x)*rstd + bg)
            for gt in range(2):
                pss = [qps.tile([128, 512], F32, tag="proj", name=f"gp{c}")
                       for c in range(QC)]
                for dt in range(DT):
                    for c4 in range(QC):
                        nc.tensor.matmul(pss[c4][:], wg_sb[:, dt, ts(gt, 128)],
                                         xt[:, dt, ts(c4, 512)],
                                         start=(dt == 0), stop=False)
                for c4 in range(QC):
                    p0, cb = 64 * (c4 % 2), c4 // 2
                    nc.tensor.matmul(pss[c4][:],
                                     ws_sb[p0:p0 + 1, ds(768 + gt * 128, 128)],
                                     mu_q[p0:p0 + 1, cb, :],
                                     start=False, stop=True)
                    gz = gtmp.tile([128, 512], F32)
                    nc.vector.tensor_tensor(out=gz[:], in0=pss[c4][:],
                                            in1=rstd_b[:, ts(c4, 512)],
                                            op=OP.mult)
                    nc.scalar.activation(g_sb[:, gt, ts(c4, 512)], gz[:],
                                         AF.Sigmoid, bias=bg_sb[:, gt:gt + 1],
                                         scale=1.0)
            # head-B gate halves moved to partitions 0:64 (SBUF->SBUF DMA)
            for pr in range(2):
                nc.sync.dma_start(gB_sb[0:64, pr, :], g_sb[64:128, pr, :])

    # ---- Phase 3: attention + o_proj ---------------------------------------
    att = ExitStack()
    bias_pool = att.enter_context(tc.tile_pool(name="bias", bufs=10))
    sps_pool = att.enter_context(tc.tile_pool(name="sps", bufs=2, space="PSUM"))
    op_pool = att.enter_context(tc.tile_pool(name="ops", bufs=1, space="PSUM"))
    yp_pool = att.enter_context(tc.tile_pool(name="yps", bufs=2, space="PSUM"))
    p_pool = att.enter_context(tc.tile_pool(name="pexp", bufs=3))
    row_pool = att.enter_context(tc.tile_pool(name="rows2", bufs=2))
    out_pool = att.enter_context(tc.tile_pool(name="outsb", bufs=2))

    biasT = io["biasT"]
    out_p = io["out_p"]

    def emit_epilogue(c4, pair, ycps):
        qlo = c4 * 512
        rb = op_pool.tile([128, 1024], F32, tag="ops", name="rb")
        for h in range(2):
            nc.tensor.matmul(rb[0:64, ts(h, 512)], ones_fr[64:65, 0:64],
                             ycps[h][64:65, :],
                             start=True, stop=True, skip_group_check=True)
        for h in range(2):
            gsl = (g_sb if h == 0 else gB_sb)[0:64, pair, ds(qlo, 512)]
            rcp = row_pool.tile([128, 512], F32, tag="rcp")
            nc.vector.reciprocal_approx_fast(out=rcp[0:64, :],
                                             in_=rb[0:64, ts(h, 512)])
            geff = row_pool.tile([128, 512], F32, tag="geff")
            nc.gpsimd.tensor_tensor(out=geff[0:64, :], in0=gsl,
                                    in1=rcp[0:64, :], op=OP.mult)
            if h == 0:
                nc.gpsimd.tensor_tensor(out=yg[0:64, pair, ds(qlo, 512)],
                                        in0=ycps[h][0:64, :],
                                        in1=geff[0:64, :], op=OP.mult)
            else:
                ygt = row_pool.tile([128, 512], BF16, tag="ygt")
                nc.gpsimd.tensor_tensor(out=ygt[0:64, :],
                                        in0=ycps[h][0:64, :],
                                        in1=geff[0:64, :], op=OP.mult)
                nc.gpsimd.dma_start(yg[64:128, pair, ds(qlo, 512)], ygt[0:64, :])

    def emit_oproj(c4):
        for i in range(4):
            nt = c4 * 4 + i
            ps = op_pool.tile([128, 1024], F32, tag="ops")
            for pt in range(2):
                for half in range(2):
                    nc.tensor.matmul(ps[:, ts(half, 512)],
                                     yg[:, pt, ts(nt, 128)],
                                     wo_sb[:, pt, ds(half * 512, 512)],
                                     start=(pt == 0), stop=(pt == 1))
            ot = out_pool.tile([128, 1024], F32)
            if i % 2 == 0:
                nc.vector.tensor_copy(ot[:], ps[:])
            else:
                nc.scalar.copy(ot[:], ps[:])
            nc.gpsimd.dma_start(out_p[ds(nt * 128, 128), :], ot[:])

    pending = []        # deferred epilogues
    done_pairs = {}     # c4 -> epilogues emitted
    def pop_pending():
        c4p, pairp, ycpsp = pending.pop(0)
        emit_epilogue(c4p, pairp, ycpsp)
        done_pairs[c4p] = done_pairs.get(c4p, 0) + 1
        if done_pairs[c4p] == 2:
            emit_oproj(c4p)

    for c4 in range(QC):
        qlo = c4 * 512
        for pair in range(2):
            yps = [yp_pool.tile([128, 512], F32, tag="yp", name=f"yp{h}")
                   for h in range(2)]
            for kt in range(KT):
                bt = bias_pool.tile([128, 2, 512], BF16, tag="bt")
                nc.sync.dma_start(bt[:], biasT[c4, pair, kt])
                s_ps = sps_pool.tile([128, 1024], F32, tag="sps")
                pe_inject = (kt % 3 == 2)
                if pe_inject:
                    for h in range(2):
                        nc.tensor.matmul(
                            s_ps[:, ts(h, 512)], id_bf[:],
                            bt[:, h, :],
                            start=True, stop=False, skip_group_check=True)
                else:
                    nc.vector.tensor_copy(s_ps[:], bt[:])
                for h, base in ((0, 0), (1, 64)):
                    nc.tensor.matmul(
                        s_ps[:, ts(h, 512)],
                        kT[base:base + 64, pair, ts(kt, 128)],
                        qT[base:base + 64, pair, ds(qlo, 512)],
                        start=False, stop=True, skip_group_check=True)
                p_t = p_pool.tile([128, 1024], BF16, tag="pt")
                nc.scalar.activation(p_t[:], s_ps[:], AF.Exp,
                                     bias=ml_sb[:, kt:kt + 1])
                for h in range(2):
                    nc.tensor.matmul(yps[h][0:65, :],
                                     v2[:, kt, pair, ds(h * 65, 65)],
                                     p_t[:, ts(h, 512)],
                                     start=(kt == 0), stop=(kt == KT - 1),
                                     skip_group_check=True)
            # free the PSUM accumulators: copy [y|den] to SBUF
            ycps = []
            for h in range(2):
                ycp = row_pool.tile([128, 512], F32R, tag="ycp", bufs=4, name="ycp")
                nc.vector.tensor_copy(ycp[0:65, :], yps[h][0:65, :])
                ycps.append(ycp)
            pending.append((c4, pair, ycps))
            if len(pending) > 1:
                pop_pending()
    while pending:
        pop_pending()
    att.close()


_CACHED = {}


def build_program(KT=8, aug=False):
    key = (KT, aug)
    if key in _CACHED:
        return _CACHED[key]
    KVN = KT * 128
    nc = bacc.Bacc("TRN2", target_bir_lowering=False, debug=False,
                   enable_asserts=False, num_devices=NCORES)
    io = {
        "xT": nc.dram_tensor("xT", (D, N), BF16, kind="ExternalInput").ap(),
        "xkvT": nc.dram_tensor("xkvT", (D, KVN), BF16,
                               kind="ExternalInput").ap(),
        "wq": nc.dram_tensor("wq", (D, 256), BF16, kind="ExternalInput").ap(),
        "wk": nc.dram_tensor("wk", (D, 256), BF16, kind="ExternalInput").ap(),
        "wv": nc.dram_tensor("wv", (D, 256), BF16, kind="ExternalInput").ap(),
        "wg": nc.dram_tensor("wg", (D, 256), BF16, kind="ExternalInput").ap(),
        "wo": nc.dram_tensor("wo", (256, D), BF16, kind="ExternalInput").ap(),
        "ws": nc.dram_tensor("ws", (128, 1024), F32R, kind="ExternalInput").ap(),
        "bg": nc.dram_tensor("bg", (128, 2), F32, kind="ExternalInput").ap(),
        "ml": nc.dram_tensor("ml", (128, KT), F32, kind="ExternalInput").ap(),
        "biasT": nc.dram_tensor("biasT", (4, 2, KT, 128, 2, 512), BF16,
                                kind="ExternalInput").ap(),
        "identf": nc.dram_tensor("identf", (128, 128), F32,
                                 kind="ExternalInput").ap(),
        "identb": nc.dram_tensor("identb", (128, 128), BF16,
                                 kind="ExternalInput").ap(),
        "out_p": nc.dram_tensor("out_p", (N, D), F32, kind="ExternalOutput").ap(),
    }
    if aug:
        io["qb"] = nc.dram_tensor("qb", (128, 2), F32, kind="ExternalInput").ap()
        io["kb"] = nc.dram_tensor("kb", (128, 2), F32, kind="ExternalInput").ap()
        io["vb"] = nc.dram_tensor("vb", (1, 256), F32R, kind="ExternalInput").ap()
    with tile.TileContext(nc) as tc, ExitStack() as ctx:
        _emit(tc, ctx, io, KT, aug)
    nc.compile()
    _CACHED[key] = nc
    return nc


def prep_in_maps(x, bias, mask, ln_w, ln_b, w_qkv, w_o, b_o, w_g, b_g):
    """Host-side sharding: slice/transpose/gather/cast only (plus exact
    folds of ln_w / ln_b / q-scale into weights, which are O(params))."""
    x = np.asarray(x, np.float32)
    bias = np.asarray(bias, np.float32)
    mask = np.asarray(mask)
    ln_w = np.asarray(ln_w, np.float32)
    ln_b = np.asarray(ln_b, np.float32)
    w_qkv = np.asarray(w_qkv, np.float32)
    w_o = np.asarray(w_o, np.float32)
    w_g = np.asarray(w_g, np.float32)
    b_g = np.asarray(b_g, np.float32)

    # key compaction (pure gather; masked keys contribute exactly zero)
    perms, kus = [], []
    for b in range(B):
        idx = np.flatnonzero(mask[b] != 0)
        if len(idx) == 0:
            idx = np.array([0])
        perms.append(idx)
        kus.append(len(idx))
    KT = max(1, (max(kus) + 127) // 128)
    KVN = KT * 128
    perm_pad = [np.concatenate([p, np.full(KVN - len(p), p[0], np.int64)])
                for p in perms]

    wql = w_qkv * ln_w[None, :]          # ln_w fold (exact)
    wgl = w_g * ln_w[None, :]
    qkv_lb = w_qkv @ ln_b                # ln_b rank-1 corrections
    g_lb = w_g @ ln_b
    aug = bool(np.any(ln_b != 0))
    qscale = np.float32(C ** -0.5)
    identf = np.eye(128, dtype=np.float32)
    bf = ml_dtypes.bfloat16

    in_maps = []
    for core in range(NCORES):
        b = core // 4
        h0 = HPC * (core % 4)
        pp = perm_pad[b]
        ku = kus[b]
        # row selections: per pair [A(64) | B(64)]
        q_rows, k_rows, v_rows = [], [], []
        for pr in range(2):
            hA, hB = h0 + 2 * pr, h0 + 2 * pr + 1
            for h in (hA, hB):
                q_rows.extend(range(h * 192, h * 192 + 64))
                k_rows.extend(range(h * 192 + 64, h * 192 + 128))
                v_rows.extend(range(h * 192 + 128, h * 192 + 192))
        q_rows, k_rows, v_rows = map(np.array, (q_rows, k_rows, v_rows))
        d0 = 64 * h0

        wq_c = np.ascontiguousarray((wql[q_rows] * qscale).T)
        wk_c = np.ascontiguousarray(wql[k_rows].T)
        wv_c = np.ascontiguousarray(wql[v_rows].T)
        wg_c = np.ascontiguousarray(wgl[d0:d0 + 256].T)
        wo_c = np.ascontiguousarray(w_o[:, d0:d0 + 256].T)
        ws = np.concatenate([-wq_c.sum(0), -wk_c.sum(0), -wv_c.sum(0),
                             -wg_c.sum(0)]).reshape(1, 1024)
        ws4 = np.ascontiguousarray(np.repeat(ws, 128, axis=0))
        bg_c = np.ascontiguousarray(
            (b_g + g_lb)[d0:d0 + 256].reshape(2, 128).T)
        ml = np.zeros(KVN, np.float32)
        ml[ku:] = NEG
        ml_c = np.ascontiguousarray(ml.reshape(KT, 128).T)
        # bias: gather keys, then [c4, pair, kt, kpart, head, q512]
        bb = bias[b, h0:h0 + 4][:, :, pp]                 # [4, 2048, KVN]
        bb = bb.reshape(2, 2, 4, 512, KT, 128)           # [pr, hd, c4, q, kt, kp]
        biasT_c = np.ascontiguousarray(
            bb.transpose(2, 0, 4, 5, 1, 3)).astype(bf)
        xT_c = np.ascontiguousarray(x[b].T).astype(bf)
        xkvT_c = np.ascontiguousarray(x[b].T[:, pp]).astype(bf)

        im = {
            "xT": xT_c, "xkvT": xkvT_c,
            "wq": wq_c.astype(bf), "wk": wk_c.astype(bf),
            "wv": wv_c.astype(bf), "wg": wg_c.astype(bf),
            "wo": wo_c.astype(bf), "ws": ws4,
            "bg": bg_c, "ml": ml_c, "biasT": biasT_c, "identf": identf,
            "identb": identf.astype(bf),
        }
        if aug:
            im["qb"] = np.ascontiguousarray(
                (qkv_lb[q_rows] * qscale).reshape(2, 128).T.astype(np.float32))
            im["kb"] = np.ascontiguousarray(
                qkv_lb[k_rows].reshape(2, 128).T.astype(np.float32))
            im["vb"] = np.ascontiguousarray(
                qkv_lb[v_rows].reshape(1, 256).astype(np.float32))
        in_maps.append(im)
    return in_maps


def gather(results, b_o):
    b_o = np.asarray(b_o, np.float32)
    out = np.zeros((B, N, D), np.float32)
    for core, res in enumerate(results):
        out[core // 4] += res["out_p"]
    out += b_o[None, None, :]
    return out


def run(inputs, **spmd_kwargs):
    from concourse import bass_utils
    in_maps = prep_in_maps(**inputs)
    KT = in_maps[0]["ml"].shape[1]
    nc = build_program(KT=KT, aug="qb" in in_maps[0])
    res = bass_utils.run_bass_kernel_spmd(
        nc, in_maps, core_ids=list(range(NCORES)), **spmd_kwargs)
    return gather(res.results, inputs["b_o"]), res


def kernel(**inputs) -> np.ndarray:
    out, _ = run(inputs)
    return out
